# revision 35
# baseline (speedup 1.0000x reference)
"""Multi-head attention Trainium2 kernel (8-core SPMD, tensor-parallel heads).

Sharding: batch b = core//4 (2 batches x 4 cores), head group g = core%4
(4 heads of 64 dims each = 256 head-dims per core). W_Q/W_K/W_V column-parallel,
W_O row-parallel; host sums the 4 partial outputs per batch (row-parallel unshard).

Inputs are pre-transposed on host (QT/KT/VT = X^T per batch) so the contraction
dim (d_model) lands on SBUF partitions without on-device transposes.

Per-core device program (all big matmuls in float32r: 1 cycle/row at free
dim >= 256, ~tf32 precision, fp32 PSUM accumulate):
  v  = VT^T @ Wv_g   (S, 260) t-major, per-head 65-wide blocks: 64 v-dims
                     plus a ones column that makes the AV matmul emit the
                     softmax denominator in PSUM row 64 for free
  kT = (Wk_g)^T @ KT + bk_g   (256, S) d'-major
  qT = (Wq_g)^T @ QT + bq_g   streamed through the attention phase, one
                              i-block ahead, in scores-PSUM slots
  attention, i-major units (head h, i-block of 512):
    scoresT[j,i] = kT_h-slice^T @ qT_h  (PSUM, j-tiles in groups of 3/2,
                   two 3-bank pools alternating; K=64 row at partition 64
                   for odd heads via tile_position)
    expT = exp(scoresT/8)  (ACT, PSUM->SBUF fused; max-subtraction elided —
                   scores are O(5) so fp32 exp is exact enough)
    av[0:65] += [v_h | 1]^T @ expT  (PSUM accumulate over j; deferred-av
                   emission keeps the in-order PE queue from stalling on exp)
    evict av: rows 0-63 -> attnu (odd heads partition-shifted via DMA),
    1/row-64 -> den0r[h] via DVE reciprocal + DMA partition shuffle
  epilogue, streamed per 512-token chunk:
    bcast 1/denom via head-indicator matmul, normalize on DVE into fresh
    tiles, out = attn_n^T @ Wo_g (+ bo/4) -> partial (S, D), host-summed.
"""

import numpy as np

import concourse.bass as bass
import concourse.bacc as bacc
import concourse.tile as tile
from concourse import mybir
from contextlib import ExitStack

P = 128
B, S_FULL, D_MODEL = 2, 2048, 1024
NUM_HEADS, DK = 16, 64
NCORES = 8
CORES_PER_BATCH = 4
HPC = NUM_HEADS // CORES_PER_BATCH  # heads per core = 4
DHC = HPC * DK  # head dims per core = 256
VW = HPC * (DK + 1)  # v storage width per t-tile = 260

F32 = mybir.dt.float32
F32R = mybir.dt.float32r
EXP = mybir.ActivationFunctionType.Exp


def _j_groups(njt):
    """Split j-tiles into groups of 3 (even count) so the two 3-bank scores
    PSUM pools alternate continuously across units with no boundary stall."""
    out, rem = [], njt
    while rem > 4:
        out.append(3)
        rem -= 3
    if rem == 4:
        out += [2, 2]
    elif rem:
        out.append(rem)
    if len(out) % 2:
        out[-1] -= 1
        out.append(1)
        out = [g for g in out if g > 0]
    return out


def build_nc(S=S_FULL, D=D_MODEL, with_bo=True, with_bv=True):
    NJT = S // P  # j tiles (16)
    NIB = S // 512  # i blocks (4)
    NKT = D // P  # d_model contraction tiles (8)
    NMT = S // P  # output token tiles (16)
    NTC = S // 512  # t chunks for projection loads (4)
    groups = _j_groups(NJT)

    nc = bacc.Bacc(trn_type="TRN2", target_bir_lowering=False, debug=False)

    qt_d = nc.dram_tensor("qt", [D, S], F32R, kind="ExternalInput")
    kt_d = nc.dram_tensor("kt", [D, S], F32R, kind="ExternalInput")
    vt_d = nc.dram_tensor("vt", [D, S], F32R, kind="ExternalInput")
    wq_d = nc.dram_tensor("wq", [D, DHC], F32R, kind="ExternalInput")
    wk_d = nc.dram_tensor("wk", [D, DHC], F32R, kind="ExternalInput")
    wv_d = nc.dram_tensor("wv", [D, DHC], F32R, kind="ExternalInput")
    wo_d = nc.dram_tensor("wo", [DHC, D], F32R, kind="ExternalInput")
    bq_d = nc.dram_tensor("bq", [DHC], F32, kind="ExternalInput")
    bk_d = nc.dram_tensor("bk", [DHC], F32, kind="ExternalInput")
    bv_d = nc.dram_tensor("bv", [DHC], F32, kind="ExternalInput")
    bo_d = nc.dram_tensor("bo", [D], F32R, kind="ExternalInput")
    ind_d = nc.dram_tensor("ind", [HPC, HPC * DK], F32R, kind="ExternalInput")
    ones_d = nc.dram_tensor("ones", [1, P], F32R, kind="ExternalInput")
    vones_d = nc.dram_tensor("vones", [P, HPC], F32R, kind="ExternalInput")
    out_d = nc.dram_tensor("out", [S, D], F32, kind="ExternalOutput")

    qt_r = qt_d.ap().rearrange("(k p) t -> p k t", p=P)
    kt_r = kt_d.ap().rearrange("(k p) t -> p k t", p=P)
    vt_r = vt_d.ap().rearrange("(k p) t -> p k t", p=P)
    wq_r = wq_d.ap().rearrange("(k p) n -> p k n", p=P)
    wk_r = wk_d.ap().rearrange("(k p) n -> p k n", p=P)
    wv_r = wv_d.ap().rearrange("(k p) n -> p k n", p=P)
    wo_r = wo_d.ap().rearrange("(k p) n -> k p n", p=P)
    out_r = out_d.ap().rearrange("(m p) n -> m p n", p=P)

    with tile.TileContext(nc) as tc, ExitStack() as octx:
        # ---- persistent pools (outer scope) ----
        qk_pool = octx.enter_context(tc.tile_pool(name="qk", bufs=2 * (DHC // P) * 2))
        v_pool = octx.enter_context(tc.tile_pool(name="v", bufs=NJT))
        au_pool = octx.enter_context(tc.tile_pool(name="attnu", bufs=DHC // P))
        den_pool = octx.enter_context(tc.tile_pool(name="den0", bufs=1))
        wo_pool = octx.enter_context(tc.tile_pool(name="wo", bufs=DHC // P))
        misc_pool = octx.enter_context(tc.tile_pool(name="misc", bufs=1))

        # qT/kT: (DHC//P) tiles of (128, S) each, d'-major
        qT = [qk_pool.tile([P, S], F32R, tag="qk", name="qk") for _ in range(DHC // P)]
        kT = [qk_pool.tile([P, S], F32R, tag="qk", name="qk") for _ in range(DHC // P)]
        # v: per t-tile (128, 260): head h at cols [65h, 65h+64], col 65h+64 = 1.0
        v_t = [v_pool.tile([P, VW], F32R, tag="v", name="v") for _ in range(NJT)]
        # unnormalized attn output, d'-major
        attnu = [au_pool.tile([P, S], F32R, tag="attnu", name="attnu") for _ in range(DHC // P)]
        # per-head reciprocal softmax denominators, head h on partition h
        den0r = den_pool.tile([HPC, S], F32R, tag="den0")

        # ---- phase B: projections ----
        with tc.tile_pool(name="x", bufs=3) as x_pool, \
             tc.tile_pool(name="w", bufs=2) as w_pool, \
             tc.tile_pool(name="pps", bufs=2, space="PSUM") as proj_ps:

            # v first (attention needs all of it), then k, then q.
            # weights: one DMA -> (128, NKT, DHC); x: one DMA per 512-chunk.
            wv_sb = w_pool.tile([P, NKT, DHC], F32R, tag="w", name="w")
            nc.sync.dma_start(wv_sb[:], wv_r)
            ones_sb = misc_pool.tile([1, P], F32R, tag="ones")
            nc.sync.dma_start(ones_sb[:], ones_d.ap())
            vone_sb = misc_pool.tile([P, HPC], F32R, tag="vones")
            nc.sync.dma_start(vone_sb[:], vones_d.ap())
            # head-indicator weights for denominator broadcast: ind[k, h*64+m] = (k==h)
            # (host-provided constant; DVE memset can't target partitions 1..3)
            ind_sb = misc_pool.tile([HPC, HPC * DK], F32R, tag="ind")
            nc.sync.dma_start(ind_sb[:], ind_d.ap())
            bq_sb = misc_pool.tile([P, DHC // P], F32, tag="bq")
            nc.sync.dma_start(bq_sb[:], bq_d.ap().rearrange("(m p) -> p m", p=P))
            bk_sb = misc_pool.tile([P, DHC // P], F32, tag="bk")
            nc.sync.dma_start(bk_sb[:], bk_d.ap().rearrange("(m p) -> p m", p=P))
            bv_sb = misc_pool.tile([P, DHC // P], F32, tag="bv")
            nc.sync.dma_start(bv_sb[:], bv_d.ap().rearrange("(m p) -> p m", p=P))
            bo_sb = misc_pool.tile([1, D], F32R, tag="bo")
            nc.sync.dma_start(bo_sb[:], bo_d.ap().rearrange("(o n) -> o n", o=1))

            wo_tiles = []
            for kk in range(DHC // P):
                t = wo_pool.tile([P, D], F32R, tag="wo")
                nc.sync.dma_start(t[:], wo_r[kk])
                wo_tiles.append(t)
            for tch in range(NTC):
                xt = x_pool.tile([P, NKT, 512], F32R, tag="x", name="x")
                nc.sync.dma_start(xt[:], vt_r[:, :, tch * 512:(tch + 1) * 512])
                for sub in range(4):
                    m = tch * 4 + sub
                    ps = proj_ps.tile([P, 512], F32, tag="pps")
                    for k in range(NKT):
                        nc.tensor.matmul(
                            ps[:, :DHC],
                            lhsT=xt[:, k, sub * P:(sub + 1) * P].bitcast(F32R),
                            rhs=wv_sb[:, k, :].bitcast(F32R),
                            start=(k == 0), stop=(k == NKT - 1),
                        )
                    # ones cols (strided memset) then per-head v blocks (one
                    # strided copy: (128, 4, 64) view of both sides)
                    vv = v_t[m][:].rearrange("p (h w) -> p h w", h=HPC)
                    nc.vector.tensor_copy(
                        vv[:, :, DK:DK + 1],
                        vone_sb[:].rearrange("p (h w) -> p h w", w=1),
                    )
                    nc.vector.tensor_copy(
                        vv[:, :, 0:DK],
                        ps[:, :DHC].rearrange("p (h w) -> p h w", h=HPC),
                    )

            # k projection (kT must be complete before any attention unit)
            wk_sb = w_pool.tile([P, NKT, DHC], F32R, tag="w", name="w")
            nc.sync.dma_start(wk_sb[:], wk_r)
            for n in range(S // 512):
                xt = x_pool.tile([P, NKT, 512], F32R, tag="x", name="x")
                nc.sync.dma_start(xt[:], kt_r[:, :, n * 512:(n + 1) * 512])
                for m in range(DHC // P):
                    ps = proj_ps.tile([P, 512], F32, tag="pps")
                    for k in range(NKT):
                        nc.tensor.matmul(
                            ps[:],
                            lhsT=wk_sb[:, k, m * P:(m + 1) * P].bitcast(F32R),
                            rhs=xt[:, k, :].bitcast(F32R),
                            start=(k == 0), stop=(k == NKT - 1),
                        )
                    nc.vector.tensor_scalar_add(
                        kT[m][:, n * 512:(n + 1) * 512], ps[:],
                        bk_sb[:, m:m + 1],
                    )

        # ---- phase C: attention (i-major), q-projection streamed inside ----
        # PSUM budget (8 banks): 2 scores pools x (128, 3*512) = 6 banks
        # + av double-buffered = 2 banks. The av matmuls of group g are
        # emitted AFTER the scores matmuls of group g+1 so the in-order PE
        # queue never stalls behind exp[g] (deferred-av emission). The q
        # projection for i-block n+1 borrows scores-pool slots and is spread
        # between the units of i-block n.
        with tc.tile_pool(name="x", bufs=2) as x_pool, \
             tc.tile_pool(name="w", bufs=1) as w_pool, \
             tc.tile_pool(name="sca", bufs=1, space="PSUM") as sc_a, \
             tc.tile_pool(name="scb", bufs=1, space="PSUM") as sc_b, \
             tc.tile_pool(name="avp", bufs=2, space="PSUM") as av_pool, \
             tc.tile_pool(name="exp", bufs=3) as exp_pool, \
             tc.tile_pool(name="drs", bufs=2) as drs_pool, \
             tc.tile_pool(name="oddt", bufs=2) as odd_pool:

            sc_pools = (sc_a, sc_b)
            gmax = max(groups)
            state = {"tgl": 0}
            pend = []  # deferred emitters (av matmuls, evicts) — cross-unit

            def sc_tile():
                t = sc_pools[state["tgl"]].tile(
                    [P, gmax * 512], F32, tag="sc", name="sc")
                state["tgl"] ^= 1
                return t

            otiles = {}
            for h in range(1, HPC, 2):
                otiles[h] = odd_pool.tile([DK, S], F32R, tag="oddt", name="oddt")

            def flush():
                while pend:
                    pend.pop(0)()

            wq_sb = w_pool.tile([P, NKT, DHC], F32R, tag="w", name="w")
            nc.sync.dma_start(wq_sb[:], wq_r)
            q_x = {}

            def q_dma(n):
                xt = x_pool.tile([P, NKT, 512], F32R, tag="x", name="x")
                nc.sync.dma_start(xt[:], qt_r[:, :, n * 512:(n + 1) * 512])
                q_x[n] = xt

            def q_proj(n, m):
                # one (m, n) chunk of the q projection in a scores-pool slot
                sc = sc_tile()
                ps = sc[:, 0:512]
                for k in range(NKT):
                    nc.tensor.matmul(
                        ps,
                        lhsT=wq_sb[:, k, m * P:(m + 1) * P].bitcast(F32R),
                        rhs=q_x[n][:, k, :].bitcast(F32R),
                        start=(k == 0), stop=(k == NKT - 1),
                    )
                nc.vector.tensor_scalar_add(
                    qT[m][:, n * 512:(n + 1) * 512], ps, bq_sb[:, m:m + 1])

            def unit(h, ib):
                th, po = h // 2, (h % 2) * DK
                tp = (po, 0) if po else None
                odd = h % 2 == 1
                otile = otiles.get(h)
                isl = slice(ib * 512, (ib + 1) * 512)
                av = av_pool.tile([P, 512], F32, tag="avp", name="av")
                jbase = 0
                for gs in groups:
                    sc = sc_tile()
                    for jj in range(gs):
                        j = jbase + jj
                        nc.tensor.matmul(
                            sc[:, jj * 512:(jj + 1) * 512],
                            lhsT=kT[th][po:po + DK, j * P:(j + 1) * P].bitcast(F32R),
                            rhs=qT[th][po:po + DK, isl].bitcast(F32R),
                            start=True, stop=True, tile_position=tp,
                        )
                    ex = exp_pool.tile([P, gmax * 512], F32R, tag="exp",
                                       name="exp")
                    nc.scalar.activation(
                        ex[:, :gs * 512], sc[:, :gs * 512], EXP, scale=0.125)
                    flush()
                    def pav(av=av, ex=ex, jbase=jbase, gs=gs, h=h):
                        for jj in range(gs):
                            j = jbase + jj
                            nc.tensor.matmul(
                                av[0:DK + 1, :],
                                lhsT=v_t[j][:, h * (DK + 1):(h + 1) * (DK + 1)].bitcast(F32R),
                                rhs=ex[:, jj * 512:(jj + 1) * 512].bitcast(F32R),
                                start=(j == 0), stop=(j == NJT - 1),
                            )
                    pend.append(pav)
                    jbase += gs

                # evict: rows 0-63 -> attnu (even heads direct, odd via DMA
                # partition shuffle); row 64 -> reciprocal into a staging
                # tile, DMA-shuffled into den0r[h] (partition h). Deferred
                # past the next unit's first scores group.
                def pev(av=av, otile=otile, odd=odd, th=th, h=h, ib=ib,
                        isl=isl):
                    if odd:
                        nc.vector.tensor_copy(otile[:, isl], av[0:DK, :])
                    else:
                        nc.vector.tensor_copy(
                            attnu[th][0:DK, isl], av[0:DK, :])
                    drs = drs_pool.tile([DK + 1, 512], F32R, tag="drs",
                                        name="drs")
                    with nc.allow_low_precision(reason="f32r is 32-bit"):
                        nc.vector.reciprocal(
                            drs[DK:DK + 1, :], av[DK:DK + 1, :])
                    nc.sync.dma_start(
                        den0r[h:h + 1, isl], drs[DK:DK + 1, :])
                    if odd and ib == NIB - 1:
                        nc.sync.dma_start(
                            attnu[th][DK:P, :], otile[:])
                pend.append(pev)

            q_dma(0)
            q_proj(0, 0)
            q_proj(0, 1)
            for n in range(NIB):
                if n + 1 < NIB:
                    q_dma(n + 1)
                for h in range(HPC):
                    unit(h, n)
                    # stream next i-block's q projection between units
                    if n + 1 < NIB and h < DHC // P:
                        q_proj(n + 1, h)
            flush()

        # ---- phase D: normalize + output projection ----
        with tc.tile_pool(name="an", bufs=6) as an_pool, \
             tc.tile_pool(name="osb", bufs=3) as out_pool, \
             tc.tile_pool(name="bcp", bufs=2, space="PSUM") as bc_pool, \
             tc.tile_pool(name="fps", bufs=3, space="PSUM") as fin_ps:

            # streamed: per 512-token chunk, normalize both th halves into
            # fresh tiles, then run the 4 output-projection m-tiles of that
            # chunk (final MMs and out-DMA overlap the next chunk's normalize)
            an_tiles = {}
            def normalize_chunk(cc):
                csl = slice(cc * 512, (cc + 1) * 512)
                for th in range(DHC // P):
                    bc = bc_pool.tile([P, 512], F32, tag="bcp")
                    # bc[m, i] = 1 / den0[2*th + m//64, i] via indicator lhsT
                    nc.tensor.matmul(
                        bc[:],
                        lhsT=ind_sb[:, th * P:(th + 1) * P].bitcast(F32R),
                        rhs=den0r[:, csl].bitcast(F32R),
                        start=True, stop=True,
                    )
                    an = an_pool.tile([P, 512], F32R, tag="an", name="an")
                    an_tiles[(th, cc)] = an
                    nc.vector.tensor_mul(an[:], attnu[th][:, csl], bc[:])
                    if with_bv:
                        # bv: per-partition (d') bias
                        nc.vector.tensor_scalar_add(
                            an[:], an[:], bv_sb[:, th:th + 1])

            normalize_chunk(0)
            for cc in range(S // 512):
                if cc + 1 < S // 512:
                    # normalize the next chunk while this chunk's projection
                    # matmuls run (keeps DVE ahead of the in-order PE queue)
                    normalize_chunk(cc + 1)
                for sub in range(4):
                    m = cc * 4 + sub
                    osb = out_pool.tile([P, D], F32, tag="osb")
                    for nn in range(D // 512):
                        ps = fin_ps.tile([P, 512], F32, tag="fps")
                        nsl = slice(nn * 512, (nn + 1) * 512)
                        for kk in range(DHC // P):
                            an = an_tiles[(kk, cc)]
                            nc.tensor.matmul(
                                ps[:],
                                lhsT=an[:, sub * P:(sub + 1) * P].bitcast(F32R),
                                rhs=wo_tiles[kk][:, nsl].bitcast(F32R),
                                start=(kk == 0),
                                stop=(not with_bo and kk == DHC // P - 1),
                            )
                        if with_bo:
                            # + bo/4 (host pre-scales bo; 4-core sum -> bo)
                            nc.tensor.matmul(
                                ps[:],
                                lhsT=ones_sb[0:1, 0:P].bitcast(F32R),
                                rhs=bo_sb[0:1, nsl].bitcast(F32R),
                                start=False, stop=True,
                            )
                        # evictions alternate DVE/ACT to halve the copy path
                        if (m * 2 + nn) % 2 == 0:
                            nc.vector.tensor_copy(osb[:, nsl], ps[:])
                        else:
                            nc.scalar.copy(osb[:, nsl], ps[:])
                    nc.sync.dma_start(out_r[m], osb[:])

    nc.compile()
    return nc


_NC_CACHE = {}


def _get_nc(S=S_FULL, D=D_MODEL, with_bo=True, with_bv=True):
    key = (S, D, with_bo, with_bv)
    if key not in _NC_CACHE:
        _NC_CACHE[key] = build_nc(S, D, with_bo, with_bv)
    return _NC_CACHE[key]


def make_in_maps(Q, K, V, Wq, bq, Wk, bk, Wv, bv, Wo, bo):
    """Host-side sharding: per-core input dict (transposes + head-slices)."""
    f32 = lambda a: np.ascontiguousarray(np.asarray(a), dtype=np.float32)
    Q, K, V = f32(Q), f32(K), f32(V)
    Wq, Wk, Wv, Wo = f32(Wq), f32(Wk), f32(Wv), f32(Wo)
    bq, bk, bv, bo = f32(bq), f32(bk), f32(bv), f32(bo)
    qt = [np.ascontiguousarray(Q[b].T) for b in range(B)]
    kt = [np.ascontiguousarray(K[b].T) for b in range(B)]
    vt = [np.ascontiguousarray(V[b].T) for b in range(B)]
    in_maps = []
    for c in range(NCORES):
        b, g = c // CORES_PER_BATCH, c % CORES_PER_BATCH
        csl = slice(g * DHC, (g + 1) * DHC)
        in_maps.append({
            "qt": qt[b], "kt": kt[b], "vt": vt[b],
            "wq": np.ascontiguousarray(Wq[:, csl]),
            "wk": np.ascontiguousarray(Wk[:, csl]),
            "wv": np.ascontiguousarray(Wv[:, csl]),
            "wo": np.ascontiguousarray(Wo[csl, :]),
            "bq": np.ascontiguousarray(bq[csl]),
            "bk": np.ascontiguousarray(bk[csl]),
            "bv": np.ascontiguousarray(bv[csl]),
            "bo": bo / CORES_PER_BATCH,
            "ind": np.repeat(np.eye(HPC, dtype=np.float32), DK, axis=1),
            "ones": np.ones((1, P), np.float32),
            "vones": np.ones((P, HPC), np.float32),
        })
    return in_maps


def kernel(Q, K, V, Wq, bq, Wk, bk, Wv, bv, Wo, bo):
    from concourse import bass_utils

    nc = _get_nc(with_bo=bool(np.any(np.asarray(bo))),
                 with_bv=bool(np.any(np.asarray(bv))))
    in_maps = make_in_maps(Q, K, V, Wq, bq, Wk, bk, Wv, bv, Wo, bo)
    res = bass_utils.run_bass_kernel_spmd(nc, in_maps, core_ids=list(range(NCORES)))
    outs = [r["out"] for r in res.results]
    full = np.zeros((B, S_FULL, D_MODEL), np.float32)
    for b in range(B):
        acc = outs[b * CORES_PER_BATCH].astype(np.float32)
        for g in range(1, CORES_PER_BATCH):
            acc = acc + outs[b * CORES_PER_BATCH + g]
        full[b] = acc
    return full
